# revision 1
# baseline (speedup 1.0000x reference)
"""Bezier-stroke rasterizer (AIR/Guide-style) as a Trainium2 Bass/Tile kernel.

Math per (batch, stroke): control points -> Bezier curve -> gaussian blob
rasterization summed over curve samples -> presence gating -> max-norm ->
tanh-norm -> sum over strokes -> tanh-norm.

Key ideas vs a direct translation:
- The 500-step curve sampling is hugely oversampled for sigma=0.03 (adjacent
  samples are ~0.1 sigma apart).  We use T=125 samples with trapezoid-matched
  weights (interior w=499/124, endpoints (1+w)/2), which reproduces the
  500-term sum to ~1e-3 relative.  One 125-partition contraction chunk.
- dx[t,(pair,g)] = curve - grid comes from ONE bf16 matmul per coordinate at
  1 cyc/row: operands are hi/lo split (c = bf16(c) + residual, -g = bf16(-g)
  + residual) into separate contraction rows, so every row is either
  bf16-exact or tiny and the reduced-precision matmul loses nothing.
- The gaussian is ONE activation pass per coordinate: Derivative_Erf(x) =
  (2/sqrt(pi)) exp(-x^2), so E = DErf(sqrt(inv)*dx) replaces the
  square+exp pair; the constant folds into the eps term of the max-norm
  (ezi *= 4/pi) and the per-t trapezoid weight is a per-partition DVE
  multiply on E_y that overlaps the x activation.  A dummy tanh reading Ex
  hoists the set17->set0 activation-table reload (1283ns) off the Tb path.
- The activations write bf16; stroke maps are 32 garbage-free [28,28] bf16
  matmuls into
  disjoint regions of one PSUM bank (strokes live at partition bases {0,32} x
  column halves) -> no masking anywhere in the epilogue.
- max-norm uses the DVE 32x32 stream transpose (no PE/DRAM round-trips);
  presence gating uses r = 1/(max + eps/z_pres) (eps/z_pres host-precomputed),
  expanded to a full [64,448] gating tensor by a single bf16 matmul against a
  constant selector.
- Zero bf16 matmuls warm the PE HAM clock gate (1.2 -> 2.4 GHz after ~3us of
  activity) during the input-DMA wait and bridge the idle gap across the
  gaussian-activation chain so the stroke matmuls run at full clock.
- The output leaves the device as bf16 (half the output-DMA payload); the
  host applies the final 1/tanh(slope) scale and the float32 cast.

Sharding: pure data parallel, 8 batches per core across 8 NeuronCores.
"""

import sys
import numpy as np
from math import comb, tanh

sys.path.insert(0, "/opt/trn_rl_repo")

from concourse import bass, bacc, tile, mybir  # noqa: E402
from concourse.bass_utils import run_bass_kernel_spmd  # noqa: E402

BS, K, PTS, RES = 64, 4, 5, 28
STEPS_REF = 500
T = 125                     # curve samples on device
NCORES = 8
BL = BS // NCORES           # local batches per core = 8
NPAIR = BL * K              # (batch, stroke) pairs per core = 32
CON = 2 * NPAIR + 2         # dx-matmul contraction rows = 66
W = NPAIR * RES             # 896 = columns per coordinate
NH = 2                      # stroke column-halves (j//2)
SP = 64                     # stroke partition extent (2 x 32-padded)
EPS = 1e-6
F32 = mybir.dt.float32
BF16 = mybir.dt.bfloat16
AF = mybir.ActivationFunctionType
ALU = mybir.AluOpType


def _host_consts():
    import ml_dtypes
    grid = np.linspace(0.0, 1.0, RES, dtype=np.float64)
    # dx[t,(pair,g)] = c_hi + c_lo - g, with every term either bf16-exact or
    # tiny, so the fp32r (reduced-precision) matmul loses nothing:
    # rows 0:32   c_hi   x  delta(p,pair)
    # rows 32:64  c_lo   x  delta(p,pair)
    # row  64     ones   x  (-g)_hi      (bf16-exact)
    # row  65     ones   x  (-g)_lo      (tiny residual)
    Q = np.zeros((CON, W), dtype=np.float32)
    for p in range(NPAIR):
        Q[p, p * RES : (p + 1) * RES] = 1.0
        Q[NPAIR + p, p * RES : (p + 1) * RES] = 1.0
    gh = (-grid).astype(ml_dtypes.bfloat16).astype(np.float64)
    gl = -grid - gh
    Q[CON - 2, :] = np.tile(gh, NPAIR)
    Q[CON - 1, :] = np.tile(gl, NPAIR)
    # deltaH[(h'*8+g'), (h, g, x)] = delta(h,h')*delta(g,g')
    deltaH = np.kron(np.eye(NH * BL, dtype=np.float32),
                     np.ones((1, RES), np.float32))             # [16, 448]
    # ksum64[(j2*32+y), y'] = delta(y,y') for y<28, zero pad rows
    ksum = np.zeros((SP, RES), dtype=np.float32)
    for j2 in range(2):
        ksum[j2 * 32 : j2 * 32 + RES] = np.eye(RES, dtype=np.float32)
    return Q, deltaH, ksum


def _basis(steps):
    t = np.linspace(0.0, 1.0, steps, dtype=np.float64)[:, None]
    i = np.arange(PTS, dtype=np.float64)[None, :]
    binom = np.array([comb(PTS - 1, j) for j in range(PTS)], dtype=np.float64)[None, :]
    return binom * (t ** i) * ((1.0 - t) ** (PTS - 1 - i))      # [steps, 5]


def _build_program(sigma, slope_strk, slope):
    inv = 1.0 / (2.0 * sigma * sigma)
    post1 = 1.0 / tanh(slope_strk)
    post2 = 1.0 / tanh(slope)

    nc = bacc.Bacc(None, target_bir_lowering=False)

    # qmat (cols 0:896) and lhs2 (cols 896:1146) packed: one critical DMA
    qz_d = nc.dram_tensor("qlhs", [CON, W + 2 * T], BF16, kind="ExternalInput")
    dl_d = nc.dram_tensor("deltah", [NH * BL, NH * BL * RES], BF16,
                          kind="ExternalInput")
    ks_d = nc.dram_tensor("ksum", [SP, RES], BF16, kind="ExternalInput")
    ez_d = nc.dram_tensor("ezw", [T, 3], F32, kind="ExternalInput")
    out_d = nc.dram_tensor("out", [BL, RES, RES], BF16, kind="ExternalOutput")

    with tile.TileContext(nc) as tc:
        with (
            tc.tile_pool(name="const", bufs=1) as cpool,
            tc.tile_pool(name="work", bufs=1) as wpool,
            tc.tile_pool(name="dsq", bufs=2, space="PSUM") as dpool,
            tc.tile_pool(name="sp", bufs=1, space="PSUM") as spool,
        ):
            # ---- input / const DMAs (critical pack first) ----
            qlhs = cpool.tile([CON, W + 2 * T], BF16)
            nc.sync.dma_start(qlhs[:], qz_d[:])
            qmat = qlhs[:, 0:W]
            lhs2 = qlhs[:, W : W + 2 * T]
            deltaH = cpool.tile([NH * BL, NH * BL * RES], BF16)
            nc.sync.dma_start(deltaH[:], dl_d[:])
            ksum = cpool.tile([SP, RES], BF16)
            nc.sync.dma_start(ksum[:], ks_d[:])
            ezw = cpool.tile([T, 3], F32)
            nc.sync.dma_start(ezw[:], ez_d[:])
            ezi = ezw[0 : NH * BL, 0:2]
            wvec = ezw[:, 2:3]

            # ---- PE warmup: the HAM clock gate releases (1.2 -> 2.4 GHz)
            # after ~3us of sustained PE activity.  Fill the DMA wait with
            # zero matmuls so the real matmuls run at full clock.
            scratch = wpool.tile([128, 512], BF16)
            nc.vector.memset(scratch[:], 0.0)
            zbias = wpool.tile([128, 1], F32)
            nc.vector.memset(zbias[:], 0.0)
            tansc = wpool.tile([1, 8], F32)
            warm = spool.tile([128, 512], F32, tag="warm")
            for _ in range(3):
                nc.tensor.matmul(
                    warm[:], scratch[:, 0:128], scratch[:],
                    start=True, stop=True, skip_group_check=True,
                )

            # ---- dx matmuls (bf16, 1 cyc/row) + gaussian activation ----
            # PSUM banks hold 512 fp32: split the 896 columns at 512/384 so
            # each matmul output stays within one bank.  Derivative_Erf IS a
            # gaussian: d/dx erf(x) = (2/sqrt(pi)) exp(-x^2), so one pass
            # computes exp(-inv*dx^2) up to a constant that the max-norm
            # absorbs (folded into ezi).  y first, so the DVE trapezoid-
            # weight multiply on Ey overlaps the x activation.
            sqinv = float(np.sqrt(inv))
            Es = []
            for coord in (1, 0):
                d2 = dpool.tile([T, W], F32, tag="d2")
                lhs = lhs2[:, coord * T : (coord + 1) * T]
                for lo, hi in ((0, 512), (512, W)):
                    nc.tensor.matmul(
                        d2[:, lo:hi], lhs, qmat[:, lo:hi],
                        start=True, stop=True,
                    )
                E = wpool.tile([T, BL, K, RES], BF16, name=f"E{coord}")
                nc.scalar.activation(
                    E[:].rearrange("p a b c -> p (a b c)"), d2[:],
                    AF.Derivative_Erf, bias=zbias[0:T, :], scale=sqinv)
                if coord == 1:
                    nc.vector.tensor_scalar_mul(
                        E[:].rearrange("p a b c -> p (a b c)"),
                        E[:].rearrange("p a b c -> p (a b c)"), wvec[:, 0:1])
                Es.append(E)
            Ey, Ex = Es
            # dummy tanh: hoists the set17->set0 activation-table reload
            # (1283ns) into the Act idle window instead of right before Tb.
            # Reads Ex so the scheduler keeps it after the Derivative_Erfs.
            nc.scalar.activation(tansc[:], Ex[0:1, 0, 0, 0:8], AF.Tanh,
                                 bias=zbias[0:1, :])

            # keep the PE clock warm across the exp wait (idle > ~3.4us
            # drops the HAM clock gate back to 1.2 GHz)
            for _ in range(8):
                nc.tensor.matmul(
                    warm[:], scratch[:, 0:128], scratch[:],
                    start=True, stop=True, skip_group_check=True,
                )

            # ---- stroke maps: 32 garbage-free [28,28] matmuls ----
            # stroke (g, j) -> partition base 32*(j%2), column (j//2, g, :).
            # Pad rows are zeroed once so downstream reductions stay clean.
            S_all = spool.tile([SP, NH, BL, RES], F32, tag="S")
            nc.vector.memset(S_all[:], 0.0)
            for h in range(NH):
                for g in range(BL):
                    for j in (2 * h, 2 * h + 1):
                        nc.tensor.matmul(
                            S_all[32 * (j % 2) : 32 * (j % 2) + RES, j // 2, g, :],
                            Ey[:, g, j, :], Ex[:, g, j, :],
                            start=True, stop=True,
                        )

            # ---- per-stroke max (max-norm) ----
            # RM is 32-padded so the partition-dim max can use the DVE
            # 32x32-block stream transpose (no PE round-trip).
            RM = wpool.tile([SP, 32], F32)
            nc.vector.memset(RM[:], 0.0)
            nc.vector.reduce_max(
                RM[:, 0 : NH * BL].rearrange("p (a b) -> p a b", b=BL),
                S_all[:], axis=mybir.AxisListType.X)
            # S -> SBUF copy overlaps the max/reciprocal chain (Act engine)
            S_sb = wpool.tile([SP, NH * BL * RES], F32)
            nc.scalar.activation(
                S_sb[:], S_all[:].rearrange("p a b c -> p (a b c)"), AF.Copy)
            RMT = wpool.tile([SP, 32], F32)
            nc.vector.transpose(RMT[:], RM[:])
            m2 = wpool.tile([NH * BL, 2], F32)
            nc.vector.reduce_max(m2[:, 0:1], RMT[0 : NH * BL, :],
                                 axis=mybir.AxisListType.X)
            nc.vector.reduce_max(m2[:, 1:2], RMT[32 : 32 + NH * BL, :],
                                 axis=mybir.AxisListType.X)
            # r = 1 / (m + eps/zp)  ==  zp / (zp*m + eps)
            r2 = wpool.tile([NH * BL, 2], F32)
            nc.vector.tensor_tensor(r2[:], m2[:], ezi[:], op=ALU.add)
            nc.vector.reciprocal(r2[:], r2[:])
            rT = wpool.tile([NH * BL, 2, 32], BF16)
            nc.vector.memset(rT[:], 0.0)
            nc.vector.tensor_copy(
                rT[:, :, 0:RES], r2[:, :, None].broadcast_to([NH * BL, 2, RES]))
            R_all = spool.tile([SP, NH * BL * RES], F32, tag="ep")
            nc.tensor.matmul(
                R_all[:], rT[:].rearrange("p a b -> p (a b)"), deltaH[:],
                start=True, stop=True,
            )

            # ---- gate, tanh-norm, stroke-sum, final tanh-norm ----
            gated = wpool.tile([SP, NH * BL * RES], F32)
            nc.vector.tensor_tensor(gated[:], R_all[:], S_sb[:], op=ALU.mult)
            Tb = wpool.tile([SP, NH, BL * RES], BF16)
            nc.scalar.activation(
                Tb[:].rearrange("p a b -> p (a b)"), gated[:],
                AF.Tanh, bias=zbias[0:SP, :], scale=float(slope_strk),
            )
            kp = spool.tile([RES, BL * RES], F32, tag="ep")
            nc.tensor.matmul(kp[:], ksum[:], Tb[:, 0, :], start=True, stop=False)
            nc.tensor.matmul(kp[:], ksum[:], Tb[:, 1, :], start=False, stop=True)
            at = wpool.tile([RES, BL, RES], BF16)
            nc.scalar.activation(
                at[:].rearrange("p b x -> p (b x)"), kp[:],
                AF.Tanh, bias=zbias[0:RES, :], scale=float(slope) * post1,
            )
            nc.sync.dma_start(out_d[:].rearrange("b y x -> y b x"), at[:])

    nc.compile()
    return nc


_CACHE = {}


def _get_program(sigma, slope_strk, slope):
    key = (float(sigma), float(slope_strk), float(slope))
    if key not in _CACHE:
        _CACHE[key] = _build_program(*key)
    return _CACHE[key]


def _host_inputs(z_pres, z_what, z_where, sigma):
    """Per-core input dicts: tiny curve linear algebra done host-side."""
    import ml_dtypes

    inv = 1.0 / (2.0 * sigma * sigma)
    Q, deltaH, ksum = _host_consts()
    Bm = _basis(T)                                            # [125, 5]
    A = (STEPS_REF - 1) / (T - 1)
    w = np.full(T, A, dtype=np.float64)
    w[0] = w[-1] = (1.0 + A) / 2.0
    wvec = w.astype(np.float32)[:, None]                      # [125, 1]

    s = z_where[..., 0].astype(np.float64)
    shift = z_where[..., 1:3].astype(np.float64)
    pts = z_what.astype(np.float64) * s[..., None, None] + shift[..., None, :]
    curve = np.einsum("tp,bkpd->bktd", Bm, pts)               # [64,4,125,2]

    consts = {
        "deltah": deltaH.astype(ml_dtypes.bfloat16),
        "ksum": ksum.astype(ml_dtypes.bfloat16),
    }
    in_maps = []
    for c in range(NCORES):
        sl = slice(c * BL, (c + 1) * BL)
        cv = curve[sl].reshape(NPAIR, T, 2)                   # [32,125,2]
        lhs2 = np.zeros((CON, 2 * T), dtype=np.float32)
        for coord in range(2):
            cT = cv[:, :, coord]                              # [32,125]
            ch = cT.astype(ml_dtypes.bfloat16).astype(np.float64)
            o = coord * T
            lhs2[0:NPAIR, o : o + T] = ch
            lhs2[NPAIR : 2 * NPAIR, o : o + T] = cT - ch
            lhs2[2 * NPAIR : 2 * NPAIR + 2, o : o + T] = 1.0
        # ezw: cols 0-1 = (4/pi)*EPS/zp in (h*8+g, j2) layout; col 2 = w
        zp = z_pres[sl].astype(np.float64)                    # [8,4]
        ezw = np.zeros((T, 3), dtype=np.float32)
        for h in range(NH):
            for g in range(BL):
                for j2 in range(2):
                    ezw[h * BL + g, j2] = (4.0 / np.pi) * EPS / max(
                        zp[g, 2 * h + j2], 1e-30)
        ezw[:, 2] = wvec[:, 0]
        m = dict(consts)
        qlhs = np.concatenate([Q, lhs2], axis=1)
        m["qlhs"] = np.ascontiguousarray(qlhs).astype(ml_dtypes.bfloat16)
        m["ezw"] = ezw
        in_maps.append(m)
    return in_maps


def kernel(z_pres, z_what, z_where, sigma, slope_strk, slope):
    z_pres = np.asarray(z_pres, np.float32)
    z_what = np.asarray(z_what, np.float32)
    z_where = np.asarray(z_where, np.float32)
    nc = _get_program(float(sigma), float(slope_strk), float(slope))
    in_maps = _host_inputs(z_pres, z_what, z_where, float(sigma))
    res = run_bass_kernel_spmd(nc, in_maps, core_ids=list(range(NCORES)))
    out = np.concatenate([np.asarray(r["out"], np.float32)
                          for r in res.results], axis=0)            # [64,28,28]
    out = out * np.float32(1.0 / tanh(float(slope)))   # post-scale on host
    return out[:, None].astype(np.float32)

